# revision 7
# baseline (speedup 1.0000x reference)
"""Trainium2 Bass kernel for nn_BinaryEncoding (per-position top-16 mask
along the 256-filter dim of [32, 256, 56, 56] activations).

Algorithm (exact threshold selection): per position row (256 channel
values in the free dim):
    max8 -> top-8 values m1 (descending), t8 = m1[7]
    "remove" the top-8, then max8 again -> m2; the 16th largest of x
    follows; mask encoded as y with (y <= 0) <=> (x >= t16).

The removal step (s2) has three engine routes, chosen per block by
`s2_pat` so every engine stays busy (the baseline ran everything on DVE,
100% saturated, with GpSimd idle):
  'V': DVE fused scalar_tensor_tensor v = (x < t8)*x   (zeroes the top-8;
       zeros rank below x_(16) since x_(16) > 0 for 256 N(0,1) w.p. ~1);
       t16 = max8(v)[7] exactly.
  'R': ScalarE activation v = Reciprocal(t8p - x), t8p = t8*(1-2^-22)
       (just below t8). Kept x < t8p map to positive v, monotone
       increasing in x; the top-8 (x >= t8p > t8p) map negative. So
       max8(v)[7] = v(x_(16)) and t16 = t8p - 1/v16 (DVE reciprocal,
       batched across the m-group). Error ~ (t8-t16)*spline_eps ~ 1e-8,
       far below inter-order-statistic gaps.
  'G': GpSimd two ops: ind = (x < t8); v = ind * x. (The fused stt is
       rejected by the backend on Pool.)

Mask (s4) routes via `s4_pat`:
  'S': ScalarE Sign(t16 - x) reading x from PSUM (ACT's fast port)
  'V': DVE tensor_scalar (x < t16) in {0,1}
  'G': GpSimd tensor_scalar (x < t16)
All encode masked <=> (y <= 0); host decodes uniformly.

Layout: HBM in is channel-major [img, 256, 3136] f32; tiles are DMA'd as
[128 ch, pos] (contiguous), transposed on TensorE via identity matmul
into PSUM [128 pos, 256 ch] in groups of `s0_batch` blocks, copied to
SBUF by one batched ScalarE Copy per group. The mask is stored pos-major
[pos, 256] fp8 (no output transposes; one batched DMA per chunk); the
host casts to f32, decodes (y <= 0) and transposes back to channel-major.

Sharding: pure data parallel, 4 images per core across 8 cores.
"""

import numpy as np

import concourse.bacc as bacc
import concourse.bass as bass
import concourse.mybir as mybir
from concourse import tile
from concourse.bass_utils import run_bass_kernel_spmd
from concourse.masks import make_identity

P = 128
C = 256                      # filter dim
N_CORES = 8
T8P_SCALE = 1.0 - 2.0 ** -22


def _segments(s, e, hw):
    """Split flat-position range [s, e) into per-image contiguous pieces.

    Returns [(img, h0, h1, off)] with off the offset inside the chunk."""
    res = []
    off = 0
    while s < e:
        img = s // hw
        h0 = s - img * hw
        h1 = min(e - img * hw, hw)
        res.append((img, h0, h1, off))
        off += h1 - h0
        s = img * hw + h1
    return res


def _strip_self_waits(nc, engines=("DVE",)):
    """Remove semaphore waits where an instruction waits on its OWN
    engine's semaphore. Engines execute their stream in order, so a wait
    on a value that only earlier same-engine instructions increment is
    always satisfied; it just adds a sem round-trip to every dispatch."""
    n = 0
    for blk in nc.m.functions[0].blocks:
        for inst in blk.instructions:
            eng = str(getattr(inst, "engine", ""))
            si = getattr(inst, "sync_info", None)
            if si is None or not si.on_wait:
                continue
            eng_name = eng.split(".")[-1]
            if eng_name not in engines:
                continue
            keep = [w for w in si.on_wait
                    if not (w.ant_name or "").startswith(eng_name + "_")]
            if len(keep) != len(si.on_wait):
                n += len(si.on_wait) - len(keep)
                si.on_wait = keep
    return n


def _plan(nblk, chunk_blocks, taper):
    if taper and nblk >= 24:
        # small first/last chunks shrink the DMA ramp at kernel start/end
        plan = [2, 2, 4, 8]
        while sum(plan) + chunk_blocks <= nblk - 6:
            plan.append(chunk_blocks)
        rem = nblk - sum(plan)
        if rem > 4:
            plan.extend([rem - 2, 2])
        elif rem > 0:
            plan.append(rem)
    else:
        assert nblk % chunk_blocks == 0
        plan = [chunk_blocks] * (nblk // chunk_blocks)
    assert sum(plan) == nblk
    return plan


def build_nc(n_img=4, hw=3136, chunk_blocks=14, in_bufs=3, out_bufs=3,
             x_bufs=3, v_bufs=6, m_bufs=5, i_bufs=4, ps_bufs=3, taper=True,
             out_dt="fp8", s2_pat="V", s4_pat="S", sign_src="psum",
             s0_batch=4, m_batch=2, d2=2, d3=3, s4_depth=5, prefetch=16,
             strip_self_waits=()):
    tot = n_img * hw
    assert tot % P == 0
    nblk = tot // P
    plan = _plan(nblk, chunk_blocks, taper)
    f32 = mybir.dt.float32
    odt = {"bf16": mybir.dt.bfloat16, "fp8": mybir.dt.float8e4,
           "f32": f32}[out_dt]

    # s2 engines must be uniform within an m-group when 'R' is used
    # (recovery is emitted per-group for 'R').
    if "R" in s2_pat:
        assert len(s2_pat) % m_batch == 0
        assert all(len(set(s2_pat[i:i + m_batch])) == 1
                   for i in range(0, len(s2_pat), m_batch)), s2_pat
        assert d2 >= m_batch, "t8p needs the whole m-group's m1"

    nc = bacc.Bacc("TRN2", target_bir_lowering=False, debug=False,
                   num_devices=N_CORES)
    x = nc.declare_dram_parameter("x", [n_img, C, hw], f32, isOutput=False)
    y = nc.declare_dram_parameter("y", [nblk, P, C], odt, isOutput=True)

    # chunk id / index-in-chunk / chunk start block for each global block
    chunk_of, j_of, c0_of = [], [], []
    b0 = 0
    for ci, cb in enumerate(plan):
        for j in range(cb):
            chunk_of.append(ci)
            j_of.append(j)
            c0_of.append(b0)
        b0 += cb

    # s0 groups: consecutive blocks within a chunk, size <= s0_batch.
    # m-groups: size <= m_batch, never crossing an s0 group.
    g0_of, jg_of, gs_of = [0] * nblk, [0] * nblk, [0] * nblk
    mg0_of, jm_of, ms_of = [0] * nblk, [0] * nblk, [0] * nblk
    for ci, cb in enumerate(plan):
        c0 = sum(plan[:ci])
        j = 0
        while j < cb:
            gs = min(s0_batch, cb - j)
            for t in range(gs):
                g0_of[c0 + j + t] = c0 + j
                jg_of[c0 + j + t] = t
                gs_of[c0 + j + t] = gs
            k = 0
            while k < gs:
                ms = min(m_batch, gs - k)
                for t in range(ms):
                    mg0_of[c0 + j + k + t] = c0 + j + k
                    jm_of[c0 + j + k + t] = t
                    ms_of[c0 + j + k + t] = ms
                k += ms
            j += gs

    def pat(p, g):
        return p[g % len(p)]

    with tile.TileContext(nc) as tc:
        with (
            tc.tile_pool(name="const", bufs=1) as const_pool,
            tc.tile_pool(name="inp", bufs=in_bufs) as in_pool,
            tc.tile_pool(name="outp", bufs=out_bufs) as out_pool,
            tc.tile_pool(name="xx", bufs=x_bufs) as x_pool,
            tc.tile_pool(name="vv", bufs=v_bufs) as v_pool,
            tc.tile_pool(name="ii", bufs=i_bufs) as i_pool,
            tc.tile_pool(name="m8", bufs=m_bufs) as m_pool,
            tc.tile_pool(name="psin", bufs=ps_bufs, space="PSUM") as ps_pool,
        ):
            ident = const_pool.tile([P, P], f32)
            make_identity(nc, ident)

            ctx = {}        # per-inflight-block state, keyed by global blk
            chunk_tiles = {}
            out_tiles = {}
            chunk_starts = [c0 for c0, j in zip(c0_of, j_of) if j == 0]

            def chunk_setup(ci):
                cb = plan[ci]
                s = chunk_starts[ci] * P
                in_lo = in_pool.tile([P, cb * P], f32, tag="in_lo")
                in_hi = in_pool.tile([P, cb * P], f32, tag="in_hi")
                if ci == 0:
                    # per-block sub-DMAs: first transpose starts sooner
                    for b in range(cb):
                        sl = slice(b * P, (b + 1) * P)
                        nc.sync.dma_start(out=in_lo[:, sl],
                                          in_=x[0, 0:P, b * P:(b + 1) * P])
                        nc.sync.dma_start(out=in_hi[:, sl],
                                          in_=x[0, P:C, b * P:(b + 1) * P])
                else:
                    for (img, h0, h1, off) in _segments(s, s + cb * P, hw):
                        n = h1 - h0
                        nc.sync.dma_start(out=in_lo[:, off:off + n],
                                          in_=x[img, 0:P, h0:h1])
                        nc.sync.dma_start(out=in_hi[:, off:off + n],
                                          in_=x[img, P:C, h0:h1])
                chunk_tiles[ci] = (in_lo, in_hi)

            def s0(g):
                if jg_of[g]:
                    return
                gs = gs_of[g]
                in_lo, in_hi = chunk_tiles[chunk_of[g]]
                # constant shapes per tag so PSUM stays at ps_bufs tiles
                ps = ps_pool.tile([P, s0_batch, C], f32, tag="ps", name="ps")
                xg = x_pool.tile([P, s0_batch, C], f32, tag="x", name="x")
                split = chunk_of[g] == 0   # fastest possible first max8
                for k in range(gs):
                    j = j_of[g + k]
                    sl = slice(j * P, (j + 1) * P)
                    nc.tensor.transpose(ps[:, k, 0:P], in_lo[:, sl], ident)
                    nc.tensor.transpose(ps[:, k, P:C], in_hi[:, sl], ident)
                    if split:
                        nc.scalar.activation(
                            xg[:, k, :], ps[:, k, :],
                            mybir.ActivationFunctionType.Copy)
                if not split:
                    nc.scalar.activation(xg[:, 0:gs, :], ps[:, 0:gs, :],
                                         mybir.ActivationFunctionType.Copy)
                for k in range(gs):
                    ctx[g + k] = {"x": xg[:, k, :], "psx": ps[:, k, :]}

            def s1(g):
                b = ctx[g]
                if jm_of[g] == 0:
                    ms = ms_of[g]
                    b["m1g"] = m_pool.tile([P, ms, 8], f32, tag=f"m1{ms}",
                                           name="m1")
                    if pat(s2_pat, g) == "R":
                        b["t8pg"] = m_pool.tile([P, ms, 1], f32,
                                                tag=f"t8p{ms}", name="t8p")
                else:
                    b["m1g"] = ctx[mg0_of[g]]["m1g"]
                    if pat(s2_pat, g) == "R":
                        b["t8pg"] = ctx[mg0_of[g]]["t8pg"]
                nc.vector.max(out=b["m1g"][:, jm_of[g], :], in_=b["x"])
                if pat(s2_pat, g) == "R" and jm_of[g] == ms_of[g] - 1:
                    # t8p = t8 * (1 - 2^-22): strictly below t8 (t8 > 0
                    # w.p. ~1), so Reciprocal's argument never hits 0.
                    g0 = mg0_of[g]
                    nc.vector.tensor_scalar(
                        out=ctx[g0]["t8pg"], in0=b["m1g"][:, :, 7:8],
                        scalar1=T8P_SCALE, scalar2=None,
                        op0=mybir.AluOpType.mult)

            def s2(g):
                b = ctx[g]
                b["v"] = v_pool.tile([P, C], f32, tag="v", name="v")
                mode = pat(s2_pat, g)
                t8 = b["m1g"][:, jm_of[g], 7:8]
                if mode == "V":
                    nc.vector.scalar_tensor_tensor(
                        out=b["v"], in0=b["x"], scalar=t8,
                        in1=b["x"], op0=mybir.AluOpType.is_lt,
                        op1=mybir.AluOpType.mult)
                elif mode == "R":
                    nc.scalar.activation(
                        b["v"], b["psx"],
                        mybir.ActivationFunctionType.Reciprocal,
                        bias=b["t8pg"][:, jm_of[g], :], scale=-1.0)
                else:  # "G"
                    ind = i_pool.tile([P, C], f32, tag="ind", name="ind")
                    nc.gpsimd.tensor_scalar(out=ind, in0=b["x"], scalar1=t8,
                                            scalar2=None,
                                            op0=mybir.AluOpType.is_lt)
                    nc.gpsimd.tensor_tensor(out=b["v"], in0=ind, in1=b["x"],
                                            op=mybir.AluOpType.mult)

            def s3(g):
                b = ctx[g]
                if jm_of[g] == 0:
                    ms = ms_of[g]
                    b["m2g"] = m_pool.tile([P, ms, 8], f32, tag=f"m2{ms}",
                                           name="m2")
                    if pat(s2_pat, g) == "R":
                        b["t16g"] = m_pool.tile([P, ms, 1], f32,
                                                tag=f"t16{ms}", name="t16")
                else:
                    b["m2g"] = ctx[mg0_of[g]]["m2g"]
                    if pat(s2_pat, g) == "R":
                        b["t16g"] = ctx[mg0_of[g]]["t16g"]
                nc.vector.max(out=b["m2g"][:, jm_of[g], :], in_=b["v"])
                if pat(s2_pat, g) == "R" and jm_of[g] == ms_of[g] - 1:
                    # t16 = t8p - (1/v16)*(1+1e-4): the 1e-4 pad biases the
                    # recovered threshold strictly BELOW x_(16) (the spline
                    # recip + fp rounding err ~1e-7 could otherwise land
                    # just above it, dropping x_(16) from the mask). The
                    # pad (~1e-5 abs) only flips positions whose
                    # x_(16)-x_(17) gap is below it: a handful in 100k.
                    g0 = mg0_of[g]
                    rv = m_pool.tile([P, ms_of[g], 1], f32,
                                     tag=f"rv{ms_of[g]}", name="rv")
                    nc.vector.reciprocal(rv, b["m2g"][:, :, 7:8])
                    nc.vector.scalar_tensor_tensor(
                        out=ctx[g0]["t16g"], in0=rv, scalar=-(1.0 + 1e-4),
                        in1=ctx[g0]["t8pg"], op0=mybir.AluOpType.mult,
                        op1=mybir.AluOpType.add)

            def s4(g):
                b = ctx[g]
                ci = chunk_of[g]
                if j_of[g] == 0:
                    outc = out_pool.tile([P, plan[ci], C], odt,
                                         tag="outc", name="outc")
                    out_tiles[ci] = outc
                outc = out_tiles[ci]
                if pat(s2_pat, g) == "R":
                    t16 = b["t16g"][:, jm_of[g], :]
                else:
                    t16 = b["m2g"][:, jm_of[g], 7:8]
                mode = pat(s4_pat, g)
                if mode == "S":
                    # Sign(t16 - x): -1/0 at selected (x >= t16), +1 below;
                    # the host decodes mask = (y <= 0).
                    src = b["psx"] if sign_src == "psum" else b["x"]
                    nc.scalar.activation(outc[:, j_of[g], :], src,
                                         mybir.ActivationFunctionType.Sign,
                                         bias=t16, scale=-1.0)
                else:
                    # (x < t16) in {0,1}: same (y <= 0) decode.
                    eng = {"V": nc.vector, "G": nc.gpsimd}[mode]
                    eng.tensor_scalar(out=outc[:, j_of[g], :], in0=b["x"],
                                      scalar1=t16, scalar2=None,
                                      op0=mybir.AluOpType.is_lt)
                if j_of[g] == plan[ci] - 1:
                    cb = plan[ci]
                    b0 = c0_of[g]
                    nc.scalar.dma_start(
                        out=y[b0:b0 + cb].rearrange("b p c -> p b c"),
                        in_=outc[:, 0:cb, :])
                    del out_tiles[ci]
                del ctx[g]

            stages = [(0, s0), (1, s1), (d2, s2), (d3, s3), (s4_depth, s4)]
            next_chunk = 0
            for step in range(nblk + s4_depth):
                while (next_chunk < len(plan)
                       and chunk_starts[next_chunk] <= step + prefetch):
                    chunk_setup(next_chunk)
                    next_chunk += 1
                for d, st in stages:
                    g = step - d
                    if 0 <= g < nblk:
                        st(g)
    nc.compile()
    if strip_self_waits:
        _strip_self_waits(nc, tuple(strip_self_waits))
    return nc


def _install_neff_cache():
    """Cache compiled NEFFs by BIR hash under /tmp so repeat runs skip
    the multi-minute neuronxcc compile."""
    import hashlib
    import os
    import shutil
    import concourse.bass2jax as b2j
    if getattr(b2j, "_topk_neff_cache_installed", False):
        return
    cache_dir = "/tmp/neff_cache"
    try:
        os.makedirs(cache_dir, exist_ok=True)
    except OSError:
        return
    orig_compile = b2j.compile_bir_kernel

    def cached_compile(ant_bir_str, compile_dir_path, neff_name):
        key = hashlib.sha256(ant_bir_str).hexdigest()[:32]
        cpath = os.path.join(cache_dir, key + ".neff")
        if os.path.exists(cpath):
            dst = os.path.join(compile_dir_path, neff_name)
            shutil.copy(cpath, dst)
            return dst
        out = orig_compile(ant_bir_str, compile_dir_path, neff_name=neff_name)
        try:
            shutil.copy(out, cpath)
        except OSError:
            pass
        return out

    b2j.compile_bir_kernel = cached_compile
    b2j._topk_neff_cache_installed = True


_install_neff_cache()

_NC_CACHE = {}


def _get_nc(n_img, hw, chunk_blocks, **kw):
    key = (n_img, hw, chunk_blocks, tuple(sorted(
        (k, str(v)) for k, v in kw.items())))
    if key not in _NC_CACHE:
        _NC_CACHE[key] = build_nc(n_img, hw, chunk_blocks, **kw)
    return _NC_CACHE[key]


KERNEL_KW = dict(s2_pat="VGGG", s4_pat="S", sign_src="psum",
                 s0_batch=4, m_batch=2, d2=2, d3=4, s4_depth=6, prefetch=24,
                 in_bufs=5, x_bufs=4, v_bufs=8, i_bufs=4, m_bufs=14,
                 ps_bufs=3, out_dt="fp8")


def make_in_maps(x, n_img, kw=KERNEL_KW):
    return [{"x": np.ascontiguousarray(x[i * n_img:(i + 1) * n_img])}
            for i in range(N_CORES)]


def unshard(core_outputs, n_img=4, hw=3136, kw=KERNEL_KW):
    """[nblk, 128, C] per-core pos-major device outputs -> [B, C, H, W].

    All mask modes encode the result so that masked <=> (y <= 0)."""
    parts = []
    for yc in core_outputs:
        yc = np.asarray(yc).reshape(n_img, hw, C).astype(np.float32)
        yc = (yc <= 0.0).astype(np.float32)
        parts.append(yc.transpose(0, 2, 1))          # -> [n_img, C, hw]
    y = np.concatenate(parts, axis=0)
    B = len(core_outputs) * n_img
    s = int(round(hw ** 0.5))
    return np.ascontiguousarray(y.reshape(B, C, s, s))


def kernel(activations: np.ndarray) -> np.ndarray:
    B, Cin, H, W = activations.shape
    assert (B, Cin, H, W) == (32, 256, 56, 56)
    hw = H * W
    n_img = B // N_CORES
    x = np.ascontiguousarray(activations, dtype=np.float32).reshape(B, Cin, hw)
    nc = _get_nc(n_img, hw, 14, **KERNEL_KW)
    in_maps = make_in_maps(x, n_img)
    res = run_bass_kernel_spmd(nc, in_maps, list(range(N_CORES)))
    return unshard([res.results[i]["y"] for i in range(N_CORES)],
                   n_img=n_img, hw=hw)


# revision 8
# speedup vs baseline: 3.3287x; 3.3287x over previous
"""Trainium2 Bass kernel for nn_BinaryEncoding (per-position top-16 mask
along the 256-filter dim of [32, 256, 56, 56] activations).

Algorithm (exact threshold selection): per position row (256 channel
values in the free dim):
    max8 -> top-8 values m1 (descending), t8 = m1[7]
    "remove" the top-8, then max8 -> m2; t16 = m2[7] = x_(16) exactly;
    mask encoded as y with (y <= 0) <=> (x >= t16).

The removal (s2) has two engine routes, chosen per block by `s2_pat`, so
the DVE (the saturated engine in the all-DVE baseline) sheds work:
  'V': DVE fused scalar_tensor_tensor v = (x < t8)*x — zeroes the top-8;
       the zeros rank below x_(16) since x_(16) > 0 for 256 N(0,1)
       samples w.p. ~1.
  'W': ScalarE sgn = Sign(t8 - x) in {-1,0,+1} (reads PSUM), then
       GpSimd v = sgn * x (tensor_tensor multiply — the only fast Pool
       op; every Pool tensor_scalar variant measures ~3.8us/block).
       Kept x < t8 pass through exactly; the top-8 flip negative (x_(8)
       itself maps to 0 via sgn=0). max8(v)[7] = x_(16) exactly.

Mask (s4) routes via `s4_pat`: 'S' ScalarE Sign(t16 - x) (PSUM src),
'V' DVE tensor_scalar (x < t16). Both encode masked <=> (y <= 0).

Layout: HBM in is channel-major [img, 256, 3136] f32; tiles are DMA'd as
[128 ch, pos] (contiguous), transposed on TensorE via identity matmul
into PSUM [128 pos, 256 ch] in groups of `s0_batch` blocks, copied to
SBUF by one batched ScalarE Copy per group. The mask output is pos-major
y[128, nblk, 256] fp8 so each s0 group DMAs out 1KB-contiguous partition
rows as soon as its masks finish (the old per-chunk channel-interleaved
DMA left a ~7us drain tail after the last compute op); the host decodes
(y <= 0) and transposes back to channel-major.

Sharding: pure data parallel, 4 images per core across 8 cores.
"""

import numpy as np

import concourse.bacc as bacc
import concourse.bass as bass
import concourse.mybir as mybir
from concourse import tile
from concourse.bass_utils import run_bass_kernel_spmd
from concourse.masks import make_identity

P = 128
C = 256                      # filter dim
N_CORES = 8


def _segments(s, e, hw):
    """Split flat-position range [s, e) into per-image contiguous pieces.

    Returns [(img, h0, h1, off)] with off the offset inside the chunk."""
    res = []
    off = 0
    while s < e:
        img = s // hw
        h0 = s - img * hw
        h1 = min(e - img * hw, hw)
        res.append((img, h0, h1, off))
        off += h1 - h0
        s = img * hw + h1
    return res


def _strip_self_waits(nc, engines=("DVE",)):
    """Remove semaphore waits where an instruction waits on its OWN
    engine's semaphore. Engines execute their stream in order, so a wait
    on a value that only earlier same-engine instructions increment is
    always satisfied; it just adds a sem round-trip to every dispatch."""
    n = 0
    for blk in nc.m.functions[0].blocks:
        for inst in blk.instructions:
            eng = str(getattr(inst, "engine", ""))
            si = getattr(inst, "sync_info", None)
            if si is None or not si.on_wait:
                continue
            eng_name = eng.split(".")[-1]
            if eng_name not in engines:
                continue
            keep = [w for w in si.on_wait
                    if not (w.ant_name or "").startswith(eng_name + "_")]
            if len(keep) != len(si.on_wait):
                n += len(si.on_wait) - len(keep)
                si.on_wait = keep
    return n


def _plan(nblk, chunk_blocks, taper):
    if taper and nblk >= 24:
        # small first/last chunks shrink the DMA ramp at kernel start/end
        plan = [2, 2, 4, 8]
        while sum(plan) + chunk_blocks <= nblk - 6:
            plan.append(chunk_blocks)
        rem = nblk - sum(plan)
        if rem > 4:
            plan.extend([rem - 2, 2])
        elif rem > 0:
            plan.append(rem)
    else:
        assert nblk % chunk_blocks == 0
        plan = [chunk_blocks] * (nblk // chunk_blocks)
    assert sum(plan) == nblk
    return plan


def build_nc(n_img=4, hw=3136, chunk_blocks=14, in_bufs=3, out_bufs=4,
             x_bufs=3, v_bufs=6, m_bufs=12, w_bufs=5, ps_bufs=3, taper=True,
             out_dt="fp8", s2_pat="V", s4_pat="S", sign_src="psum",
             s0_batch=4, d2=2, d3=4, s4_depth=6, prefetch=16,
             strip_self_waits=()):
    tot = n_img * hw
    assert tot % P == 0
    nblk = tot // P
    plan = _plan(nblk, chunk_blocks, taper)
    f32 = mybir.dt.float32
    odt = {"bf16": mybir.dt.bfloat16, "fp8": mybir.dt.float8e4,
           "f32": f32}[out_dt]

    nc = bacc.Bacc("TRN2", target_bir_lowering=False, debug=False,
                   num_devices=N_CORES)
    x = nc.declare_dram_parameter("x", [n_img, C, hw], f32, isOutput=False)
    y = nc.declare_dram_parameter("y", [P, nblk, C], odt, isOutput=True)

    # chunk id / index-in-chunk for each global block
    chunk_of, j_of = [], []
    for ci, cb in enumerate(plan):
        for j in range(cb):
            chunk_of.append(ci)
            j_of.append(j)

    # s0 groups: consecutive blocks within a chunk, size <= s0_batch
    g0_of, jg_of, gs_of = [0] * nblk, [0] * nblk, [0] * nblk
    for ci, cb in enumerate(plan):
        c0 = sum(plan[:ci])
        j = 0
        while j < cb:
            gs = min(s0_batch, cb - j)
            for t in range(gs):
                g0_of[c0 + j + t] = c0 + j
                jg_of[c0 + j + t] = t
                gs_of[c0 + j + t] = gs
            j += gs

    def pat(p, g):
        return p[g % len(p)]

    with tile.TileContext(nc) as tc:
        with (
            tc.tile_pool(name="const", bufs=1) as const_pool,
            tc.tile_pool(name="inp", bufs=in_bufs) as in_pool,
            tc.tile_pool(name="outp", bufs=out_bufs) as out_pool,
            tc.tile_pool(name="xx", bufs=x_bufs) as x_pool,
            tc.tile_pool(name="vv", bufs=v_bufs) as v_pool,
            tc.tile_pool(name="sg", bufs=w_bufs) as sg_pool,
            tc.tile_pool(name="m8", bufs=m_bufs) as m_pool,
            tc.tile_pool(name="psin", bufs=ps_bufs, space="PSUM") as ps_pool,
        ):
            ident = const_pool.tile([P, P], f32)
            make_identity(nc, ident)

            ctx = {}        # per-inflight-block state, keyed by global blk
            chunk_tiles = {}
            group_tiles = {}
            chunk_starts = []
            b0 = 0
            for cb in plan:
                chunk_starts.append(b0)
                b0 += cb

            def chunk_setup(ci):
                cb = plan[ci]
                s = chunk_starts[ci] * P
                in_lo = in_pool.tile([P, cb * P], f32, tag="in_lo")
                in_hi = in_pool.tile([P, cb * P], f32, tag="in_hi")
                if ci == 0:
                    # per-block sub-DMAs: first transpose starts sooner
                    for b in range(cb):
                        sl = slice(b * P, (b + 1) * P)
                        nc.sync.dma_start(out=in_lo[:, sl],
                                          in_=x[0, 0:P, b * P:(b + 1) * P])
                        nc.sync.dma_start(out=in_hi[:, sl],
                                          in_=x[0, P:C, b * P:(b + 1) * P])
                else:
                    for (img, h0, h1, off) in _segments(s, s + cb * P, hw):
                        n = h1 - h0
                        nc.sync.dma_start(out=in_lo[:, off:off + n],
                                          in_=x[img, 0:P, h0:h1])
                        nc.sync.dma_start(out=in_hi[:, off:off + n],
                                          in_=x[img, P:C, h0:h1])
                chunk_tiles[ci] = (in_lo, in_hi)

            def s0(g):
                if jg_of[g]:
                    return
                gs = gs_of[g]
                in_lo, in_hi = chunk_tiles[chunk_of[g]]
                ps = ps_pool.tile([P, s0_batch, C], f32, tag="ps", name="ps")
                xg = x_pool.tile([P, s0_batch, C], f32, tag="x", name="x")
                og = out_pool.tile([P, s0_batch, C], odt, tag="o", name="o")
                group_tiles[g] = og
                split = chunk_of[g] == 0   # fastest possible first max8
                for k in range(gs):
                    j = j_of[g + k]
                    sl = slice(j * P, (j + 1) * P)
                    nc.tensor.transpose(ps[:, k, 0:P], in_lo[:, sl], ident)
                    nc.tensor.transpose(ps[:, k, P:C], in_hi[:, sl], ident)
                    if split:
                        nc.scalar.activation(
                            xg[:, k, :], ps[:, k, :],
                            mybir.ActivationFunctionType.Copy)
                if not split:
                    nc.scalar.activation(xg[:, 0:gs, :], ps[:, 0:gs, :],
                                         mybir.ActivationFunctionType.Copy)
                for k in range(gs):
                    ctx[g + k] = {"x": xg[:, k, :], "psx": ps[:, k, :],
                                  "out": og}

            def s1(g):
                b = ctx[g]
                b["m1"] = m_pool.tile([P, 8], f32, tag="m1", name="m1")
                nc.vector.max(out=b["m1"], in_=b["x"])

            def s2(g):
                b = ctx[g]
                b["v"] = v_pool.tile([P, C], f32, tag="v", name="v")
                t8 = b["m1"][:, 7:8]
                if pat(s2_pat, g) == "V":
                    nc.vector.scalar_tensor_tensor(
                        out=b["v"], in0=b["x"], scalar=t8,
                        in1=b["x"], op0=mybir.AluOpType.is_lt,
                        op1=mybir.AluOpType.mult)
                else:  # "W": ScalarE sign + GpSimd multiply
                    sgn = sg_pool.tile([P, C], f32, tag="sgn", name="sgn")
                    nc.scalar.activation(sgn, b["psx"],
                                         mybir.ActivationFunctionType.Sign,
                                         bias=t8, scale=-1.0)
                    nc.gpsimd.tensor_tensor(out=b["v"], in0=sgn, in1=b["x"],
                                            op=mybir.AluOpType.mult)

            def s3(g):
                b = ctx[g]
                b["m2"] = m_pool.tile([P, 8], f32, tag="m2", name="m2")
                nc.vector.max(out=b["m2"], in_=b["v"])

            def s4(g):
                b = ctx[g]
                og = b["out"]
                t16 = b["m2"][:, 7:8]
                mode = pat(s4_pat, g)
                if mode == "S":
                    # Sign(t16 - x): -1/0 at selected (x >= t16), +1 below;
                    # the host decodes mask = (y <= 0).
                    src = b["psx"] if sign_src == "psum" else b["x"]
                    nc.scalar.activation(og[:, jg_of[g], :], src,
                                         mybir.ActivationFunctionType.Sign,
                                         bias=t16, scale=-1.0)
                else:
                    # (x < t16) in {0,1}: same (y <= 0) decode.
                    nc.vector.tensor_scalar(out=og[:, jg_of[g], :],
                                            in0=b["x"], scalar1=t16,
                                            scalar2=None,
                                            op0=mybir.AluOpType.is_lt)
                if jg_of[g] == gs_of[g] - 1:
                    g0 = g0_of[g]
                    gs = gs_of[g]
                    nc.sync.dma_start(out=y[:, g0:g0 + gs, :],
                                      in_=og[:, 0:gs, :])
                    del group_tiles[g0]
                del ctx[g]

            stages = [(0, s0), (1, s1), (d2, s2), (d3, s3), (s4_depth, s4)]
            next_chunk = 0
            for step in range(nblk + s4_depth):
                while (next_chunk < len(plan)
                       and chunk_starts[next_chunk] <= step + prefetch):
                    chunk_setup(next_chunk)
                    next_chunk += 1
                for d, st in stages:
                    g = step - d
                    if 0 <= g < nblk:
                        st(g)
    nc.compile()
    if strip_self_waits:
        _strip_self_waits(nc, tuple(strip_self_waits))
    return nc


def _install_neff_cache():
    """Cache compiled NEFFs by BIR hash under /tmp so repeat runs skip
    the multi-minute neuronxcc compile."""
    import hashlib
    import os
    import shutil
    import concourse.bass2jax as b2j
    if getattr(b2j, "_topk_neff_cache_installed", False):
        return
    cache_dir = "/tmp/neff_cache"
    try:
        os.makedirs(cache_dir, exist_ok=True)
    except OSError:
        return
    orig_compile = b2j.compile_bir_kernel

    def cached_compile(ant_bir_str, compile_dir_path, neff_name):
        key = hashlib.sha256(ant_bir_str).hexdigest()[:32]
        cpath = os.path.join(cache_dir, key + ".neff")
        if os.path.exists(cpath):
            dst = os.path.join(compile_dir_path, neff_name)
            shutil.copy(cpath, dst)
            return dst
        out = orig_compile(ant_bir_str, compile_dir_path, neff_name=neff_name)
        try:
            shutil.copy(out, cpath)
        except OSError:
            pass
        return out

    b2j.compile_bir_kernel = cached_compile
    b2j._topk_neff_cache_installed = True


_install_neff_cache()

_NC_CACHE = {}


def _get_nc(n_img, hw, chunk_blocks, **kw):
    key = (n_img, hw, chunk_blocks, tuple(sorted(
        (k, str(v)) for k, v in kw.items())))
    if key not in _NC_CACHE:
        _NC_CACHE[key] = build_nc(n_img, hw, chunk_blocks, **kw)
    return _NC_CACHE[key]


KERNEL_KW = dict(s2_pat="VWVWVWVWVWV", s4_pat="S", sign_src="psum",
                 s0_batch=4, d2=2, d3=4, s4_depth=6, prefetch=24,
                 in_bufs=5, x_bufs=4, v_bufs=8, w_bufs=5, m_bufs=14,
                 ps_bufs=3, out_bufs=4, out_dt="fp8")


def make_in_maps(x, n_img, kw=KERNEL_KW):
    return [{"x": np.ascontiguousarray(x[i * n_img:(i + 1) * n_img])}
            for i in range(N_CORES)]


def unshard(core_outputs, n_img=4, hw=3136, kw=KERNEL_KW):
    """[P, nblk, C] per-core pos-major device outputs -> [B, C, H, W].

    All mask modes encode the result so that masked <=> (y <= 0)."""
    parts = []
    for yc in core_outputs:
        yc = np.asarray(yc)                          # [P, nblk, C]
        yc = np.moveaxis(yc, 0, 1)                   # -> [nblk, P, C]
        yc = yc.reshape(n_img, hw, C).astype(np.float32)
        yc = (yc <= 0.0).astype(np.float32)
        parts.append(yc.transpose(0, 2, 1))          # -> [n_img, C, hw]
    y = np.concatenate(parts, axis=0)
    B = len(core_outputs) * n_img
    s = int(round(hw ** 0.5))
    return np.ascontiguousarray(y.reshape(B, C, s, s))


def kernel(activations: np.ndarray) -> np.ndarray:
    B, Cin, H, W = activations.shape
    assert (B, Cin, H, W) == (32, 256, 56, 56)
    hw = H * W
    n_img = B // N_CORES
    x = np.ascontiguousarray(activations, dtype=np.float32).reshape(B, Cin, hw)
    nc = _get_nc(n_img, hw, 14, **KERNEL_KW)
    in_maps = make_in_maps(x, n_img)
    res = run_bass_kernel_spmd(nc, in_maps, list(range(N_CORES)))
    return unshard([res.results[i]["y"] for i in range(N_CORES)],
                   n_img=n_img, hw=hw)
